# revision 15
# baseline (speedup 1.0000x reference)
"""Single-head attention (b=4, n=2048, d_model=1024, head=64) on 8 TRN2 cores.

Sharding: 2-way data parallel over batch x 2-way sequence parallel over
queries. Core c handles batch c//2, query half c%2.

Own-half-first overlap structure:
  - x arrives as two per-half transposed tensors; the own half is sequenced
    first on both DMA rings so q/k_own/v_own projections finish well before
    the other half lands.
  - phase 2 starts on the own k-tiles (scores -> exp) while the other half
    streams in; its k/v projection matmuls interleave into the own loop.
  - scores are unpacked single matmuls: own tiles read k_own/q_hi from SBUF
    partitions 64:128 (tile_position (64,0)), oth tiles read k_oth/q_lo from
    partitions 0:64 -- no cross-partition moves needed.
  - v reaches natural layout via one DMA-XBAR transpose per half (no PE
    transposes, no psum); own-half attn@v is deferred until that transpose
    lands, filling otherwise-idle PE slots under the exp stream.
  - attn and v run in fp8-e4m3 with DoubleRow packing (2 k-tiles per matmul)
    when FP8_AV is on; exp writes fp8 directly.
  - the device emits outT [65, 1024] = [sum(attn*v) | sum(attn)]; the host
    does the sequence-parallel combine (divide) + transpose during unshard.
"""

import os
import sys

if "/opt/trn_rl_repo" not in sys.path:
    sys.path.insert(0, "/opt/trn_rl_repo")

import numpy as np
import ml_dtypes

import concourse.bass as bass  # noqa: F401
from concourse import bacc
import concourse.mybir as mybir
import concourse.tile as tile
from concourse.bass import ts, ds
from concourse.bass_utils import run_bass_kernel_spmd

BF16 = mybir.dt.bfloat16
FP8 = mybir.dt.float8e4
F32 = mybir.dt.float32
AFT = mybir.ActivationFunctionType
DR = mybir.MatmulPerfMode.DoubleRow
NPBF16 = ml_dtypes.bfloat16

B, N, D, H = 4, 2048, 1024, 64
NCORES = 8
NQ = N // 2        # query rows per core
NCH = D // 128     # d_model chunks
AUGW = 128         # augmented-v width (dual-fp8 ldweights needs 64/128 cols)

FP8_AV = os.environ.get("FP8_AV", "0") == "1"
WARMUP = int(os.environ.get("WARMUP", "7"))


def _build_nc(fp8_av=FP8_AV):
    nc = bacc.Bacc("TRN2", target_bir_lowering=False, debug=False)

    xo_d = nc.dram_tensor("xo", [128, NCH, NQ], BF16, kind="ExternalInput")
    xa_d = nc.dram_tensor("xa", [128, NCH, NQ], BF16, kind="ExternalInput")
    wqk_d = nc.dram_tensor("wqk", [128, NCH, 128], BF16, kind="ExternalInput")
    wkv_d = nc.dram_tensor("wkv", [128, NCH, 128], BF16, kind="ExternalInput")
    wvq_d = nc.dram_tensor("wvq", [128, NCH, 128], BF16, kind="ExternalInput")
    bias_d = nc.dram_tensor("bias", [128, 3], F32, kind="ExternalInput")
    out_d = nc.dram_tensor("out", [H + 1, NQ], F32, kind="ExternalOutput")

    with tile.TileContext(nc) as tc:
        with (
            tc.tile_pool(name="const", bufs=1) as cpool,
            tc.tile_pool(name="x", bufs=1) as xpool,
            tc.tile_pool(name="main", bufs=1) as mpool,
            tc.tile_pool(name="attn", bufs=4) as apool,
        ):
            # ---- DMA issues; emission order == per-ring transfer order ----
            # sync:   bias wqk xo0 xo23 xo4 | xa01 xa45 vtA vtB out
            # scalar: wvq xo1 xo5 xo67 wkv | xa23 xa67
            xot = [None] * NCH

            def xo_load(c, sl, eng):
                t = xpool.tile([128, sl, NQ], BF16, name=f"xo{c}")
                eng.dma_start(out=t, in_=xo_d[:, c:c + sl, :])
                for k in range(sl):
                    xot[c + k] = t[:, k, :]

            xo_load(0, 1, nc.sync)
            wqk = cpool.tile([128, NCH, 128], BF16)
            nc.gpsimd.dma_start(out=wqk, in_=wqk_d.ap())
            wvq = cpool.tile([128, NCH, 128], BF16)
            nc.gpsimd.dma_start(out=wvq, in_=wvq_d.ap())
            xo_load(1, 1, nc.scalar)
            xo_load(2, 2, nc.sync)
            xo_load(5, 1, nc.scalar)
            xo_load(4, 1, nc.sync)
            xo_load(6, 2, nc.scalar)
            bias_t = cpool.tile([128, 3], F32)
            nc.gpsimd.dma_start(out=bias_t, in_=bias_d.ap())
            wkv = cpool.tile([128, NCH, 128], BF16)
            nc.gpsimd.dma_start(out=wkv, in_=wkv_d.ap())

            xat = [None] * NCH
            for c, eng in ((0, nc.sync), (2, nc.scalar), (4, nc.sync),
                           (6, nc.scalar)):
                t = xpool.tile([128, 2, NQ], BF16, name=f"xa{c}")
                eng.dma_start(out=t, in_=xa_d[:, c:c + 2, :])
                xat[c], xat[c + 1] = t[:, 0, :], t[:, 1, :]

            # persistent sbuf (vaug8 first: dual-fp8 ldweights wants an
            # aligned base)
            if fp8_av:
                vaug8 = mpool.tile([128, 16, AUGW], FP8)
            qTlo = mpool.tile([128, NQ], BF16)   # 0:64 q_lo | 64:128 k_own
            vq = mpool.tile([128, NQ], BF16)     # 0:64 v_own | 64:128 q_hi
            kvB = mpool.tile([128, NQ], BF16)    # 0:64 k_oth | 64:128 v_oth
            wu = mpool.tile([128, 512], BF16)
            nc.vector.memset(wu[:], 1.0)
            vaug = mpool.tile([128, 16, AUGW], BF16)
            nc.vector.memset(vaug[:], 1.0)
            outSb = mpool.tile([H + 1, NQ], F32)

            # ---- phase 1 (own half): projections ----
            with tc.tile_pool(name="psum1", bufs=1, space="PSUM") as pp1:
                psA = pp1.tile([128, NQ], F32)
                psC = pp1.tile([128, NQ], F32)
                wu_ps = pp1.tile([128, 512], F32)
                for _ in range(WARMUP):
                    nc.tensor.matmul(wu_ps[:], lhsT=wu[:, 0:128], rhs=wu[:],
                                     start=True, stop=True)
                corder = (0, 1, 2, 3, 4, 5, 6, 7)
                for i, c in enumerate(corder):
                    st, sp = i == 0, i == len(corder) - 1
                    for s in range(2):
                        nc.tensor.matmul(psA[:, ds(s * 512, 512)],
                                         lhsT=wqk[:, c, :],
                                         rhs=xot[c][:, ds(s * 512, 512)],
                                         start=st, stop=sp)
                    for s in range(2):
                        nc.tensor.matmul(psC[:, ds(s * 512, 512)],
                                         lhsT=wvq[:, c, :],
                                         rhs=xot[c][:, ds(s * 512, 512)],
                                         start=st, stop=sp)

                # split copies into column halves so the first scores start
                # after half a copy on each engine
                for hcol in range(2):
                    cs = ds(hcol * 512, 512)
                    nc.scalar.activation(out=qTlo[:, cs], in_=psA[:, cs],
                                         func=AFT.Identity,
                                         bias=bias_t[:, 0:1])
                    nc.vector.tensor_scalar_add(out=vq[:, cs], in0=psC[:, cs],
                                                scalar1=bias_t[:, 2:3])
                for _ in range(4):
                    nc.tensor.matmul(wu_ps[:], lhsT=wu[:, 0:128], rhs=wu[:],
                                     start=True, stop=True)

            # ---- phase 2 ----
            with tc.tile_pool(name="psum2", bufs=1, space="PSUM") as pp2:
                psB = pp2.tile([128, NQ], F32)
                outTa = pp2.tile([AUGW, 512], F32, tag="outTa")
                outTb = pp2.tile([AUGW, 512], F32, tag="outTb")
                outT = (outTa, outTb)
                navs = [0]
                nav_total = 8 if fp8_av else 16

                def scores(t, own):
                    sT = pp2.tile([128, NQ], F32, tag="sc", bufs=2)
                    if own:
                        lhsT, rhs, tp = (qTlo[64:128, ts(t, 128)],
                                         vq[64:128, :], (64, 0))
                    else:
                        lhsT, rhs, tp = (kvB[0:64, ts(t, 128)],
                                         qTlo[0:64, :], (0, 0))
                    for s in range(2):
                        nc.tensor.matmul(sT[:, ds(s * 512, 512)], lhsT=lhsT,
                                         rhs=rhs[:, ds(s * 512, 512)],
                                         start=True, stop=True,
                                         tile_position=tp)
                    return sT

                def av(j, at):
                    st, sp = navs[0] == 0, navs[0] == nav_total - 1
                    navs[0] += 1
                    for s in range(2):
                        if fp8_av:
                            nc.tensor.matmul(outT[s][:],
                                             lhsT=vaug8[:, j:j + 2, :],
                                             rhs=at[:, :, ds(s * 512, 512)],
                                             start=st, stop=sp, perf_mode=DR)
                        else:
                            nc.tensor.matmul(outT[s][:], lhsT=vaug[:, j, :],
                                             rhs=at[:, ds(s * 512, 512)],
                                             start=st, stop=sp)

                def half_loop(own, defer_av, fill=()):
                    deferred = []
                    fill = list(fill)
                    atp = None
                    for t in range(8):
                        j = t + (8 if own else 0)
                        sT = scores(t, own)
                        for j2, at2 in fill[2 * t:2 * t + 2]:
                            av(j2, at2)
                        if own:  # oth k/v projections ride along
                            for s in range(2):
                                nc.tensor.matmul(
                                    psB[:, ds(s * 512, 512)],
                                    lhsT=wkv[:, t, :],
                                    rhs=xat[t][:, ds(s * 512, 512)],
                                    start=t == 0, stop=t == 7 and s == 1)
                        if fp8_av:
                            if t % 2 == 0:
                                atp = apool.tile([128, 2, NQ], FP8, tag="at",
                                                 bufs=6)
                            nc.scalar.activation(out=atp[:, t % 2, :],
                                                 in_=sT[:], func=AFT.Exp)
                            if t % 2 == 1:
                                if defer_av:
                                    deferred.append((j - 1, atp))
                                else:
                                    av(j - 1, atp)
                        else:
                            at = apool.tile([128, NQ], BF16, tag="at", bufs=10)
                            nc.scalar.activation(out=at[:], in_=sT[:],
                                                 func=AFT.Exp)
                            if defer_av:
                                deferred.append((j, at))
                            else:
                                av(j, at)
                    return deferred

                # v_own -> natural tiles j=8..15; rides the sync ring after
                # the oth-x transfers, so own-half av is deferred
                nc.sync.dma_start_transpose(out=vaug[:, 8:16, 0:H],
                                            in_=vq[0:64, :])
                if fp8_av:
                    nc.vector.tensor_copy(out=vaug8[:, 8:16, :],
                                          in_=vaug[:, 8:16, :])

                own_avs = half_loop(own=True, defer_av=True)
                for j2, at2 in own_avs[:2]:
                    av(j2, at2)

                # oth k/v: psum -> sbuf, first quarter early so the first
                # oth scores can start
                nc.vector.tensor_scalar_add(out=kvB[:, 0:256],
                                            in0=psB[:, 0:256],
                                            scalar1=bias_t[:, 1:2])
                nc.vector.tensor_scalar_add(out=kvB[:, 256:1024],
                                            in0=psB[:, 256:1024],
                                            scalar1=bias_t[:, 1:2])
                nc.sync.dma_start_transpose(out=vaug[:, 0:8, 0:H],
                                            in_=kvB[64:128, :])
                if fp8_av:
                    nc.vector.tensor_copy(out=vaug8[:, 0:8, :],
                                          in_=vaug[:, 0:8, :])

                half_loop(own=False, defer_av=False, fill=own_avs[2:])

                # ---- epilogue: psum -> sbuf, DMA out (host divides) ----
                nc.vector.tensor_copy(out=outSb[:, 0:512],
                                      in_=outTa[0:H + 1, :])
                nc.vector.tensor_copy(out=outSb[:, 512:1024],
                                      in_=outTb[0:H + 1, :])
                nc.sync.dma_start(out=out_d.ap(), in_=outSb[:])

    nc.compile()
    return nc


_NC_CACHE = None


def _get_nc():
    global _NC_CACHE
    if _NC_CACHE is None:
        _NC_CACHE = _build_nc()
    return _NC_CACHE


def _make_in_maps(x, wq, bq, wk, bk, wv, bv):
    x = np.asarray(x, np.float32)
    wq = np.asarray(wq, np.float32)
    bq = np.asarray(bq, np.float32)
    wk = np.asarray(wk, np.float32)
    bk = np.asarray(bk, np.float32)
    wv = np.asarray(wv, np.float32)
    bv = np.asarray(bv, np.float32)

    wqs, bqs = wq / 8.0, bq / 8.0  # fold 1/sqrt(head) into q
    pack = lambda w: np.ascontiguousarray(
        w.reshape(NCH, 128, 128).transpose(1, 0, 2)).astype(NPBF16)
    shared = {
        "wqk": pack(np.concatenate([wqs, wk], 1)),
        "wkv": pack(np.concatenate([wk, wv], 1)),
        "wvq": pack(np.concatenate([wv, wqs], 1)),
        "bias": np.ascontiguousarray(np.stack(
            [np.concatenate([bqs, bk]),
             np.concatenate([bk, bv]),
             np.concatenate([bv, bqs])], 1)).astype(np.float32),
    }
    packx = lambda xm: np.ascontiguousarray(
        xm.T.reshape(NCH, 128, NQ).transpose(1, 0, 2)).astype(NPBF16)
    in_maps = []
    for c in range(NCORES):
        b, h = c // 2, c % 2
        own = x[b, h * NQ:(h + 1) * NQ]
        oth = x[b, (1 - h) * NQ:(2 - h) * NQ]
        in_maps.append({"xo": packx(own), "xa": packx(oth), **shared})
    return in_maps


def _gather(results):
    out = np.empty((B, N, H), np.float32)
    for c in range(NCORES):
        b, h = c // 2, c % 2
        o = np.asarray(results[c]["out"])  # [65, NQ]: num rows | den row
        out[b, h * NQ:(h + 1) * NQ] = (o[0:H] / o[H:H + 1]).T
    return out


def run(inputs, trace=False, tmpdir=None):
    nc = _get_nc()
    in_maps = _make_in_maps(**inputs)
    res = run_bass_kernel_spmd(nc, in_maps, list(range(NCORES)), trace=trace,
                               tmpdir=tmpdir)
    return _gather(res.results), res


def kernel(**inputs):
    out, _ = run(inputs, trace=False)
    return out


# revision 16
# speedup vs baseline: 1.1001x; 1.1001x over previous
"""Single-head attention (b=4, n=2048, d_model=1024, head=64) on 8 TRN2 cores.

Sharding: 2-way data parallel over batch x 2-way sequence parallel over
queries. Core c handles batch c//2, query half c%2.

Own-half-first overlap structure:
  - x arrives as two per-half transposed tensors; the own half is sequenced
    first on both DMA rings so q/k_own/v_own projections finish well before
    the other half lands.
  - phase 2 starts on the own k-tiles (scores -> exp) while the other half
    streams in; its k/v projection matmuls interleave into the own loop.
  - scores are unpacked single matmuls: own tiles read k_own/q_hi from SBUF
    partitions 64:128 (tile_position (64,0)), oth tiles read k_oth/q_lo from
    partitions 0:64 -- no cross-partition moves needed.
  - v reaches natural layout via one DMA-XBAR transpose per half (no PE
    transposes, no psum); own-half attn@v is deferred until that transpose
    lands, filling otherwise-idle PE slots under the exp stream.
  - attn and v run in fp8-e4m3 with DoubleRow packing (2 k-tiles per matmul)
    when FP8_AV is on; exp writes fp8 directly.
  - the device emits outT [65, 1024] = [sum(attn*v) | sum(attn)]; the host
    does the sequence-parallel combine (divide) + transpose during unshard.
"""

import os
import sys

if "/opt/trn_rl_repo" not in sys.path:
    sys.path.insert(0, "/opt/trn_rl_repo")

import numpy as np
import ml_dtypes

import concourse.bass as bass  # noqa: F401
from concourse import bacc
import concourse.mybir as mybir
import concourse.tile as tile
from concourse.bass import ts, ds
from concourse.bass_utils import run_bass_kernel_spmd

BF16 = mybir.dt.bfloat16
FP8 = mybir.dt.float8e4
F32 = mybir.dt.float32
AFT = mybir.ActivationFunctionType
DR = mybir.MatmulPerfMode.DoubleRow
NPBF16 = ml_dtypes.bfloat16

B, N, D, H = 4, 2048, 1024, 64
NCORES = 8
NQ = N // 2        # query rows per core
NCH = D // 128     # d_model chunks
AUGW = 128         # augmented-v width (dual-fp8 ldweights needs 64/128 cols)

FP8_AV = os.environ.get("FP8_AV", "0") == "1"
WARMUP = int(os.environ.get("WARMUP", "7"))


def _build_nc(fp8_av=FP8_AV):
    nc = bacc.Bacc("TRN2", target_bir_lowering=False, debug=False)

    xo_d = nc.dram_tensor("xo", [128, NCH, NQ], BF16, kind="ExternalInput")
    xa_d = nc.dram_tensor("xa", [128, NCH, NQ], BF16, kind="ExternalInput")
    wqk_d = nc.dram_tensor("wqk", [128, NCH, 128], BF16, kind="ExternalInput")
    wkv_d = nc.dram_tensor("wkv", [128, NCH, 128], BF16, kind="ExternalInput")
    wvq_d = nc.dram_tensor("wvq", [128, NCH, 128], BF16, kind="ExternalInput")
    bias_d = nc.dram_tensor("bias", [128, 3], F32, kind="ExternalInput")
    out_d = nc.dram_tensor("out", [H + 1, NQ], F32, kind="ExternalOutput")

    with tile.TileContext(nc) as tc:
        with (
            tc.tile_pool(name="const", bufs=1) as cpool,
            tc.tile_pool(name="x", bufs=1) as xpool,
            tc.tile_pool(name="main", bufs=1) as mpool,
            tc.tile_pool(name="attn", bufs=4) as apool,
        ):
            # ---- DMA issues; emission order == per-ring transfer order ----
            # sync:   bias wqk xo0 xo23 xo4 | xa01 xa45 vtA vtB out
            # scalar: wvq xo1 xo5 xo67 wkv | xa23 xa67
            xot = [None] * NCH

            def xo_load(c, sl, eng):
                t = xpool.tile([128, sl, NQ], BF16, name=f"xo{c}")
                eng.dma_start(out=t, in_=xo_d[:, c:c + sl, :])
                for k in range(sl):
                    xot[c + k] = t[:, k, :]

            xo_load(0, 1, nc.sync)
            wqk = cpool.tile([128, NCH, 128], BF16)
            nc.scalar.dma_start(out=wqk, in_=wqk_d.ap())
            wvq = cpool.tile([128, NCH, 128], BF16)
            nc.sync.dma_start(out=wvq, in_=wvq_d.ap())
            xo_load(1, 1, nc.scalar)
            xo_load(2, 2, nc.sync)
            xo_load(5, 1, nc.scalar)
            xo_load(4, 1, nc.sync)
            xo_load(6, 2, nc.scalar)
            bias_t = cpool.tile([128, 3], F32)
            nc.sync.dma_start(out=bias_t, in_=bias_d.ap())
            wkv = cpool.tile([128, NCH, 128], BF16)
            nc.scalar.dma_start(out=wkv, in_=wkv_d.ap())

            xat = [None] * NCH
            for c, eng in ((0, nc.sync), (2, nc.scalar), (4, nc.sync),
                           (6, nc.scalar)):
                t = xpool.tile([128, 2, NQ], BF16, name=f"xa{c}")
                eng.dma_start(out=t, in_=xa_d[:, c:c + 2, :])
                xat[c], xat[c + 1] = t[:, 0, :], t[:, 1, :]

            # persistent sbuf (vaug8 first: dual-fp8 ldweights wants an
            # aligned base)
            if fp8_av:
                vaug8 = mpool.tile([128, 16, AUGW], FP8)
            qTlo = mpool.tile([128, NQ], BF16)   # 0:64 q_lo | 64:128 k_own
            vq = mpool.tile([128, NQ], BF16)     # 0:64 v_own | 64:128 q_hi
            kvB = mpool.tile([128, NQ], BF16)    # 0:64 k_oth | 64:128 v_oth
            wu = mpool.tile([128, 512], BF16)
            nc.vector.memset(wu[:], 1.0)
            vaug = mpool.tile([128, 16, AUGW], BF16)
            nc.vector.memset(vaug[:], 1.0)
            outSb = mpool.tile([H + 1, NQ], F32)

            # ---- phase 1 (own half): projections ----
            with tc.tile_pool(name="psum1", bufs=1, space="PSUM") as pp1:
                psA = pp1.tile([128, NQ], F32)
                psC = pp1.tile([128, NQ], F32)
                wu_ps = pp1.tile([128, 512], F32)
                for _ in range(WARMUP):
                    nc.tensor.matmul(wu_ps[:], lhsT=wu[:, 0:128], rhs=wu[:],
                                     start=True, stop=True)
                corder = (0, 1, 2, 3, 4, 5, 6, 7)
                for i, c in enumerate(corder):
                    st, sp = i == 0, i == len(corder) - 1
                    for s in range(2):
                        nc.tensor.matmul(psA[:, ds(s * 512, 512)],
                                         lhsT=wqk[:, c, :],
                                         rhs=xot[c][:, ds(s * 512, 512)],
                                         start=st, stop=sp)
                    for s in range(2):
                        nc.tensor.matmul(psC[:, ds(s * 512, 512)],
                                         lhsT=wvq[:, c, :],
                                         rhs=xot[c][:, ds(s * 512, 512)],
                                         start=st, stop=sp)

                # split copies into column halves so the first scores start
                # after half a copy on each engine
                for hcol in range(2):
                    cs = ds(hcol * 512, 512)
                    nc.scalar.activation(out=qTlo[:, cs], in_=psA[:, cs],
                                         func=AFT.Identity,
                                         bias=bias_t[:, 0:1])
                    nc.vector.tensor_scalar_add(out=vq[:, cs], in0=psC[:, cs],
                                                scalar1=bias_t[:, 2:3])
                for _ in range(4):
                    nc.tensor.matmul(wu_ps[:], lhsT=wu[:, 0:128], rhs=wu[:],
                                     start=True, stop=True)

            # ---- phase 2 ----
            with tc.tile_pool(name="psum2", bufs=1, space="PSUM") as pp2:
                psB = pp2.tile([128, NQ], F32)
                outTa = pp2.tile([AUGW, 512], F32, tag="outTa")
                outTb = pp2.tile([AUGW, 512], F32, tag="outTb")
                outT = (outTa, outTb)
                navs = [0]
                nav_total = 8 if fp8_av else 16

                def scores(t, own):
                    sT = pp2.tile([128, NQ], F32, tag="sc", bufs=2)
                    if own:
                        lhsT, rhs, tp = (qTlo[64:128, ts(t, 128)],
                                         vq[64:128, :], (64, 0))
                    else:
                        lhsT, rhs, tp = (kvB[0:64, ts(t, 128)],
                                         qTlo[0:64, :], (0, 0))
                    for s in range(2):
                        nc.tensor.matmul(sT[:, ds(s * 512, 512)], lhsT=lhsT,
                                         rhs=rhs[:, ds(s * 512, 512)],
                                         start=True, stop=True,
                                         tile_position=tp)
                    return sT

                def av(j, at):
                    st, sp = navs[0] == 0, navs[0] == nav_total - 1
                    navs[0] += 1
                    for s in range(2):
                        if fp8_av:
                            nc.tensor.matmul(outT[s][:],
                                             lhsT=vaug8[:, j:j + 2, :],
                                             rhs=at[:, :, ds(s * 512, 512)],
                                             start=st, stop=sp, perf_mode=DR)
                        else:
                            nc.tensor.matmul(outT[s][:], lhsT=vaug[:, j, :],
                                             rhs=at[:, ds(s * 512, 512)],
                                             start=st, stop=sp)

                def half_loop(own, defer_av, fill=()):
                    deferred = []
                    fill = list(fill)
                    atp = None
                    for t in range(8):
                        j = t + (8 if own else 0)
                        sT = scores(t, own)
                        for j2, at2 in fill[2 * t:2 * t + 2]:
                            av(j2, at2)
                        if own:  # oth k/v projections ride along
                            for s in range(2):
                                nc.tensor.matmul(
                                    psB[:, ds(s * 512, 512)],
                                    lhsT=wkv[:, t, :],
                                    rhs=xat[t][:, ds(s * 512, 512)],
                                    start=t == 0, stop=t == 7 and s == 1)
                        if fp8_av:
                            if t % 2 == 0:
                                atp = apool.tile([128, 2, NQ], FP8, tag="at",
                                                 bufs=6)
                            nc.scalar.activation(out=atp[:, t % 2, :],
                                                 in_=sT[:], func=AFT.Exp)
                            if t % 2 == 1:
                                if defer_av:
                                    deferred.append((j - 1, atp))
                                else:
                                    av(j - 1, atp)
                        else:
                            at = apool.tile([128, NQ], BF16, tag="at", bufs=10)
                            nc.scalar.activation(out=at[:], in_=sT[:],
                                                 func=AFT.Exp)
                            if defer_av:
                                deferred.append((j, at))
                            else:
                                av(j, at)
                    return deferred

                # v_own -> natural tiles j=8..15; rides the sync ring after
                # the oth-x transfers, so own-half av is deferred
                nc.sync.dma_start_transpose(out=vaug[:, 8:16, 0:H],
                                            in_=vq[0:64, :])
                if fp8_av:
                    nc.vector.tensor_copy(out=vaug8[:, 8:16, :],
                                          in_=vaug[:, 8:16, :])

                own_avs = half_loop(own=True, defer_av=True)
                for j2, at2 in own_avs[:2]:
                    av(j2, at2)

                # oth k/v: psum -> sbuf, first quarter early so the first
                # oth scores can start
                nc.vector.tensor_scalar_add(out=kvB[:, 0:256],
                                            in0=psB[:, 0:256],
                                            scalar1=bias_t[:, 1:2])
                nc.vector.tensor_scalar_add(out=kvB[:, 256:1024],
                                            in0=psB[:, 256:1024],
                                            scalar1=bias_t[:, 1:2])
                nc.sync.dma_start_transpose(out=vaug[:, 0:8, 0:H],
                                            in_=kvB[64:128, :])
                if fp8_av:
                    nc.vector.tensor_copy(out=vaug8[:, 0:8, :],
                                          in_=vaug[:, 0:8, :])

                half_loop(own=False, defer_av=False, fill=own_avs[2:])

                # ---- epilogue: psum -> sbuf, DMA out (host divides) ----
                nc.vector.tensor_copy(out=outSb[:, 0:512],
                                      in_=outTa[0:H + 1, :])
                nc.vector.tensor_copy(out=outSb[:, 512:1024],
                                      in_=outTb[0:H + 1, :])
                nc.sync.dma_start(out=out_d.ap(), in_=outSb[:])

    nc.compile()
    return nc


_NC_CACHE = None


def _get_nc():
    global _NC_CACHE
    if _NC_CACHE is None:
        _NC_CACHE = _build_nc()
    return _NC_CACHE


def _make_in_maps(x, wq, bq, wk, bk, wv, bv):
    x = np.asarray(x, np.float32)
    wq = np.asarray(wq, np.float32)
    bq = np.asarray(bq, np.float32)
    wk = np.asarray(wk, np.float32)
    bk = np.asarray(bk, np.float32)
    wv = np.asarray(wv, np.float32)
    bv = np.asarray(bv, np.float32)

    wqs, bqs = wq / 8.0, bq / 8.0  # fold 1/sqrt(head) into q
    pack = lambda w: np.ascontiguousarray(
        w.reshape(NCH, 128, 128).transpose(1, 0, 2)).astype(NPBF16)
    shared = {
        "wqk": pack(np.concatenate([wqs, wk], 1)),
        "wkv": pack(np.concatenate([wk, wv], 1)),
        "wvq": pack(np.concatenate([wv, wqs], 1)),
        "bias": np.ascontiguousarray(np.stack(
            [np.concatenate([bqs, bk]),
             np.concatenate([bk, bv]),
             np.concatenate([bv, bqs])], 1)).astype(np.float32),
    }
    packx = lambda xm: np.ascontiguousarray(
        xm.T.reshape(NCH, 128, NQ).transpose(1, 0, 2)).astype(NPBF16)
    in_maps = []
    for c in range(NCORES):
        b, h = c // 2, c % 2
        own = x[b, h * NQ:(h + 1) * NQ]
        oth = x[b, (1 - h) * NQ:(2 - h) * NQ]
        in_maps.append({"xo": packx(own), "xa": packx(oth), **shared})
    return in_maps


def _gather(results):
    out = np.empty((B, N, H), np.float32)
    for c in range(NCORES):
        b, h = c // 2, c % 2
        o = np.asarray(results[c]["out"])  # [65, NQ]: num rows | den row
        out[b, h * NQ:(h + 1) * NQ] = (o[0:H] / o[H:H + 1]).T
    return out


def run(inputs, trace=False, tmpdir=None):
    nc = _get_nc()
    in_maps = _make_in_maps(**inputs)
    res = run_bass_kernel_spmd(nc, in_maps, list(range(NCORES)), trace=trace,
                               tmpdir=tmpdir)
    return _gather(res.results), res


def kernel(**inputs):
    out, _ = run(inputs, trace=False)
    return out


# revision 17
# speedup vs baseline: 1.1133x; 1.0120x over previous
"""Single-head attention (b=4, n=2048, d_model=1024, head=64) on 8 TRN2 cores.

Sharding: 2-way data parallel over batch x 2-way sequence parallel over
queries. Core c handles batch c//2, query half c%2.

Own-half-first overlap structure:
  - x arrives as two per-half transposed tensors; the own half is sequenced
    first on both DMA rings so q/k_own/v_own projections finish well before
    the other half lands.
  - phase 2 starts on the own k-tiles (scores -> exp) while the other half
    streams in; its k/v projection matmuls interleave into the own loop.
  - scores are unpacked single matmuls: own tiles read k_own/q_hi from SBUF
    partitions 64:128 (tile_position (64,0)), oth tiles read k_oth/q_lo from
    partitions 0:64 -- no cross-partition moves needed.
  - v reaches natural layout via one DMA-XBAR transpose per half (no PE
    transposes, no psum); own-half attn@v is deferred until that transpose
    lands, filling otherwise-idle PE slots under the exp stream.
  - attn and v run in fp8-e4m3 with DoubleRow packing (2 k-tiles per matmul)
    when FP8_AV is on; exp writes fp8 directly.
  - the device emits outT [65, 1024] = [sum(attn*v) | sum(attn)]; the host
    does the sequence-parallel combine (divide) + transpose during unshard.
"""

import os
import sys

if "/opt/trn_rl_repo" not in sys.path:
    sys.path.insert(0, "/opt/trn_rl_repo")

import numpy as np
import ml_dtypes

import concourse.bass as bass  # noqa: F401
from concourse import bacc
import concourse.mybir as mybir
import concourse.tile as tile
from concourse.bass import ts, ds
from concourse.bass_utils import run_bass_kernel_spmd

BF16 = mybir.dt.bfloat16
FP8 = mybir.dt.float8e4
F32 = mybir.dt.float32
AFT = mybir.ActivationFunctionType
DR = mybir.MatmulPerfMode.DoubleRow
NPBF16 = ml_dtypes.bfloat16

B, N, D, H = 4, 2048, 1024, 64
NCORES = 8
NQ = N // 2        # query rows per core
NCH = D // 128     # d_model chunks
AUGW = 128         # augmented-v width (dual-fp8 ldweights needs 64/128 cols)

FP8_AV = os.environ.get("FP8_AV", "0") == "1"
WARMUP = int(os.environ.get("WARMUP", "7"))


def _build_nc(fp8_av=FP8_AV):
    nc = bacc.Bacc("TRN2", target_bir_lowering=False, debug=False)

    xo_d = nc.dram_tensor("xo", [128, NCH, NQ], BF16, kind="ExternalInput")
    xa_d = nc.dram_tensor("xa", [128, NCH, NQ], BF16, kind="ExternalInput")
    wqk_d = nc.dram_tensor("wqk", [128, NCH, 128], BF16, kind="ExternalInput")
    wkv_d = nc.dram_tensor("wkv", [128, NCH, 128], BF16, kind="ExternalInput")
    wvq_d = nc.dram_tensor("wvq", [128, NCH, 128], BF16, kind="ExternalInput")
    bias_d = nc.dram_tensor("bias", [128, 3], F32, kind="ExternalInput")
    out_d = nc.dram_tensor("out", [H + 1, NQ], F32, kind="ExternalOutput")

    with tile.TileContext(nc) as tc:
        with (
            tc.tile_pool(name="const", bufs=1) as cpool,
            tc.tile_pool(name="x", bufs=1) as xpool,
            tc.tile_pool(name="main", bufs=1) as mpool,
            tc.tile_pool(name="attn", bufs=4) as apool,
        ):
            # ---- DMA issues; emission order == per-ring transfer order ----
            # sync:   bias wqk xo0 xo23 xo4 | xa01 xa45 vtA vtB out
            # scalar: wvq xo1 xo5 xo67 wkv | xa23 xa67
            xot = [None] * NCH

            def xo_load(c, sl, eng):
                t = xpool.tile([128, sl, NQ], BF16, name=f"xo{c}")
                eng.dma_start(out=t, in_=xo_d[:, c:c + sl, :])
                for k in range(sl):
                    xot[c + k] = t[:, k, :]

            xo_load(0, 1, nc.sync)
            wqk = cpool.tile([128, NCH, 128], BF16)
            nc.scalar.dma_start(out=wqk, in_=wqk_d.ap())
            wvq = cpool.tile([128, NCH, 128], BF16)
            nc.sync.dma_start(out=wvq, in_=wvq_d.ap())
            xo_load(1, 1, nc.scalar)
            xo_load(2, 2, nc.sync)
            xo_load(5, 1, nc.scalar)
            xo_load(4, 1, nc.sync)
            xo_load(6, 2, nc.scalar)
            bias_t = cpool.tile([128, 3], F32)
            nc.sync.dma_start(out=bias_t, in_=bias_d.ap())
            wkv = cpool.tile([128, NCH, 128], BF16)
            nc.scalar.dma_start(out=wkv, in_=wkv_d.ap())

            xat = [None] * NCH
            for c, eng in ((0, nc.sync), (2, nc.scalar), (4, nc.sync),
                           (6, nc.scalar)):
                t = xpool.tile([128, 2, NQ], BF16, name=f"xa{c}")
                eng.dma_start(out=t, in_=xa_d[:, c:c + 2, :])
                xat[c], xat[c + 1] = t[:, 0, :], t[:, 1, :]

            # persistent sbuf (vaug8 first: dual-fp8 ldweights wants an
            # aligned base)
            if fp8_av:
                vaug8 = mpool.tile([128, 16, AUGW], FP8)
            qTlo = mpool.tile([128, NQ], BF16)   # 0:64 q_lo | 64:128 k_own
            vq = mpool.tile([128, NQ], BF16)     # 0:64 v_own | 64:128 q_hi
            kvB = mpool.tile([128, NQ], BF16)    # 0:64 k_oth | 64:128 v_oth
            wu = mpool.tile([128, 512], BF16)
            nc.vector.memset(wu[:], 1.0)
            vaug = mpool.tile([128, 16, AUGW], BF16)
            nc.vector.memset(vaug[:], 1.0)
            outSb = mpool.tile([H + 1, NQ], F32)

            # ---- phase 1 (own half): projections ----
            with tc.tile_pool(name="psum1", bufs=1, space="PSUM") as pp1:
                psA = pp1.tile([128, NQ], F32)
                psC = pp1.tile([128, NQ], F32)
                wu_ps = pp1.tile([128, 512], F32)
                for _ in range(WARMUP):
                    nc.tensor.matmul(wu_ps[:], lhsT=wu[:, 0:128], rhs=wu[:],
                                     start=True, stop=True)
                corder = (0, 1, 2, 3, 4, 5, 6, 7)
                for i, c in enumerate(corder):
                    st, sp = i == 0, i == len(corder) - 1
                    for s in range(2):
                        nc.tensor.matmul(psA[:, ds(s * 512, 512)],
                                         lhsT=wqk[:, c, :],
                                         rhs=xot[c][:, ds(s * 512, 512)],
                                         start=st, stop=sp)
                    for s in range(2):
                        nc.tensor.matmul(psC[:, ds(s * 512, 512)],
                                         lhsT=wvq[:, c, :],
                                         rhs=xot[c][:, ds(s * 512, 512)],
                                         start=st, stop=sp)

                # split copies into column halves so the first scores start
                # after half a copy on each engine
                for hcol in range(2):
                    cs = ds(hcol * 512, 512)
                    nc.scalar.activation(out=qTlo[:, cs], in_=psA[:, cs],
                                         func=AFT.Identity,
                                         bias=bias_t[:, 0:1])
                    nc.vector.tensor_scalar_add(out=vq[:, cs], in0=psC[:, cs],
                                                scalar1=bias_t[:, 2:3])
                for _ in range(4):
                    nc.tensor.matmul(wu_ps[:], lhsT=wu[:, 0:128], rhs=wu[:],
                                     start=True, stop=True)

            # ---- phase 2 ----
            with tc.tile_pool(name="psum2", bufs=1, space="PSUM") as pp2:
                psB = pp2.tile([128, NQ], F32)
                outTa = pp2.tile([AUGW, 512], F32, tag="outTa")
                outTb = pp2.tile([AUGW, 512], F32, tag="outTb")
                outT = (outTa, outTb)
                navs = [0]
                nav_total = 8 if fp8_av else 16

                def scores(t, own):
                    sT = pp2.tile([128, NQ], F32, tag="sc", bufs=2)
                    if own:
                        lhsT, rhs, tp = (qTlo[64:128, ts(t, 128)],
                                         vq[64:128, :], (64, 0))
                    else:
                        lhsT, rhs, tp = (kvB[0:64, ts(t, 128)],
                                         qTlo[0:64, :], (0, 0))
                    for s in range(2):
                        nc.tensor.matmul(sT[:, ds(s * 512, 512)], lhsT=lhsT,
                                         rhs=rhs[:, ds(s * 512, 512)],
                                         start=True, stop=True,
                                         tile_position=tp)
                    return sT

                def av(j, at):
                    st, sp = navs[0] == 0, navs[0] == nav_total - 1
                    navs[0] += 1
                    for s in range(2):
                        if fp8_av:
                            nc.tensor.matmul(outT[s][:],
                                             lhsT=vaug8[:, j:j + 2, :],
                                             rhs=at[:, :, ds(s * 512, 512)],
                                             start=st, stop=sp, perf_mode=DR)
                        else:
                            nc.tensor.matmul(outT[s][:], lhsT=vaug[:, j, :],
                                             rhs=at[:, ds(s * 512, 512)],
                                             start=st, stop=sp)

                def half_loop(own, defer_av, fill=()):
                    deferred = []
                    fill = list(fill)
                    atp = None
                    for t in range(8):
                        j = t + (8 if own else 0)
                        sT = scores(t, own)
                        for j2, at2 in fill[t:t + 1]:
                            av(j2, at2)
                        if own:  # oth k/v projections ride along
                            for s in range(2):
                                nc.tensor.matmul(
                                    psB[:, ds(s * 512, 512)],
                                    lhsT=wkv[:, t, :],
                                    rhs=xat[t][:, ds(s * 512, 512)],
                                    start=t == 0, stop=t == 7 and s == 1)
                        if fp8_av:
                            if t % 2 == 0:
                                atp = apool.tile([128, 2, NQ], FP8, tag="at",
                                                 bufs=6)
                            nc.scalar.activation(out=atp[:, t % 2, :],
                                                 in_=sT[:], func=AFT.Exp)
                            if t % 2 == 1:
                                if defer_av:
                                    deferred.append((j - 1, atp))
                                else:
                                    av(j - 1, atp)
                        else:
                            at = apool.tile([128, NQ], BF16, tag="at", bufs=10)
                            nc.scalar.activation(out=at[:], in_=sT[:],
                                                 func=AFT.Exp)
                            if defer_av:
                                deferred.append((j, at))
                            else:
                                av(j, at)
                    return deferred

                # v_own -> natural tiles j=8..15; rides the sync ring after
                # the oth-x transfers, so own-half av is deferred
                nc.sync.dma_start_transpose(out=vaug[:, 8:16, 0:H],
                                            in_=vq[0:64, :])
                if fp8_av:
                    nc.vector.tensor_copy(out=vaug8[:, 8:16, :],
                                          in_=vaug[:, 8:16, :])

                own_avs = half_loop(own=True, defer_av=True)
                for j2, at2 in own_avs[:2]:
                    av(j2, at2)
                own_fill = own_avs[2:6]
                own_tail = own_avs[6:]

                # oth k/v: psum -> sbuf, first quarter early so the first
                # oth scores can start
                nc.vector.tensor_scalar_add(out=kvB[:, 0:256],
                                            in0=psB[:, 0:256],
                                            scalar1=bias_t[:, 1:2])
                nc.vector.tensor_scalar_add(out=kvB[:, 256:1024],
                                            in0=psB[:, 256:1024],
                                            scalar1=bias_t[:, 1:2])
                nc.sync.dma_start_transpose(out=vaug[:, 0:8, 0:H],
                                            in_=kvB[64:128, :])
                if fp8_av:
                    nc.vector.tensor_copy(out=vaug8[:, 0:8, :],
                                          in_=vaug[:, 0:8, :])

                half_loop(own=False, defer_av=False, fill=own_fill)
                for j2, at2 in own_tail:
                    av(j2, at2)

                # ---- epilogue: psum -> sbuf, DMA out (host divides) ----
                nc.vector.tensor_copy(out=outSb[:, 0:512],
                                      in_=outTa[0:H + 1, :])
                nc.vector.tensor_copy(out=outSb[:, 512:1024],
                                      in_=outTb[0:H + 1, :])
                nc.sync.dma_start(out=out_d.ap()[:, 0:512],
                                  in_=outSb[:, 0:512])
                nc.scalar.dma_start(out=out_d.ap()[:, 512:1024],
                                    in_=outSb[:, 512:1024])

    nc.compile()
    return nc


_NC_CACHE = None


def _get_nc():
    global _NC_CACHE
    if _NC_CACHE is None:
        _NC_CACHE = _build_nc()
    return _NC_CACHE


def _make_in_maps(x, wq, bq, wk, bk, wv, bv):
    x = np.asarray(x, np.float32)
    wq = np.asarray(wq, np.float32)
    bq = np.asarray(bq, np.float32)
    wk = np.asarray(wk, np.float32)
    bk = np.asarray(bk, np.float32)
    wv = np.asarray(wv, np.float32)
    bv = np.asarray(bv, np.float32)

    wqs, bqs = wq / 8.0, bq / 8.0  # fold 1/sqrt(head) into q
    pack = lambda w: np.ascontiguousarray(
        w.reshape(NCH, 128, 128).transpose(1, 0, 2)).astype(NPBF16)
    shared = {
        "wqk": pack(np.concatenate([wqs, wk], 1)),
        "wkv": pack(np.concatenate([wk, wv], 1)),
        "wvq": pack(np.concatenate([wv, wqs], 1)),
        "bias": np.ascontiguousarray(np.stack(
            [np.concatenate([bqs, bk]),
             np.concatenate([bk, bv]),
             np.concatenate([bv, bqs])], 1)).astype(np.float32),
    }
    packx = lambda xm: np.ascontiguousarray(
        xm.T.reshape(NCH, 128, NQ).transpose(1, 0, 2)).astype(NPBF16)
    in_maps = []
    for c in range(NCORES):
        b, h = c // 2, c % 2
        own = x[b, h * NQ:(h + 1) * NQ]
        oth = x[b, (1 - h) * NQ:(2 - h) * NQ]
        in_maps.append({"xo": packx(own), "xa": packx(oth), **shared})
    return in_maps


def _gather(results):
    out = np.empty((B, N, H), np.float32)
    for c in range(NCORES):
        b, h = c // 2, c % 2
        o = np.asarray(results[c]["out"])  # [65, NQ]: num rows | den row
        out[b, h * NQ:(h + 1) * NQ] = (o[0:H] / o[H:H + 1]).T
    return out


def run(inputs, trace=False, tmpdir=None):
    nc = _get_nc()
    in_maps = _make_in_maps(**inputs)
    res = run_bass_kernel_spmd(nc, in_maps, list(range(NCORES)), trace=trace,
                               tmpdir=tmpdir)
    return _gather(res.results), res


def kernel(**inputs):
    out, _ = run(inputs, trace=False)
    return out
